# revision 4
# baseline (speedup 1.0000x reference)
"""Trainium2 Bass kernel for nn_MFF_38809324487316 (topk_masking).

Pure data parallel: batch dim 16 -> 8 cores x 2 samples; the tiny ECA/conv/BN
params are replicated (folded into one packed constant block per core).

Per sample, the whole top-k gather / mean / 1x1-conv pipeline is one
data-dependent [256,256] @ [256,6400] matmul:
  rows   0..127 : G            -> x1[pos_idx]            (tmp1 gather rows)
  row    128    : negmask/128  -> mean of negatives      (tmp1 mean row)
  rows 129..255 : W_pos @ G + outer(w_last, negmask/128) (the 1x1 conv)
followed by BN + LeakyReLU (ACT affine + one max-based VE op; the mean row
passes through via per-partition alpha[0]=1) and a +x1 add.

The data-dependent 0/1 matrices are built on-device from the ECA channel
scores with no sort and no data-dependent control flow:
  - channel scores y for BOTH samples via two accumulating PE matmuls
    against a host-built banded matrix (GAP + ECA conv fused; sigmoid
    dropped - it is monotone so the ranking is unchanged),
  - ranks for all 4 (sample, half) pairs via fused compare+row-sum
    (tensor_scalar with accum_out),
  - gather matrix G via iota == position equality.

The big matmuls run in bf16 (x1 cast during a fused cast+channel-sum VE op
whose f32 accum_out keeps the ranking exact; G is 0/1 so only the routed x
values and conv weights see bf16 rounding - measured rel err 3.2e-03 vs the
2e-2 gate). All big transfers ride ONE HWDGE queue (nc.sync) - measured
faster on HW than splitting across queues; the x0 passthrough is DRAM->DRAM
after each sample's stores. Per-iteration HW time ~164 us/core, equal to the
measured pure-DMA floor for the same traffic (52.4 MB HBM per core per
iteration; theoretical 358 GB/s bound is 146 us).
"""

import sys

sys.path.insert(0, "/opt/trn_rl_repo")

import numpy as np

import concourse.bass as bass
import concourse.tile as tile
from concourse import mybir
from concourse.bass_utils import run_bass_kernel_spmd

B, C, H, W = 16, 256, 80, 80
HALF = C // 2           # 128
NPIX = H * W            # 6400
NCORES = 8
SPC = B // NCORES       # 2 samples per core
NT = 512                # matmul n-tile (one PSUM bank of f32)
GRP = 1024              # epilogue group (2 PSUM banks)
BN_EPS = 1e-5
F32 = mybir.dt.float32
BF16 = mybir.dt.bfloat16

# cblk column offsets
O_ID = 0
O_TRI = 128
O_ONES = 256
O_IOTA = 384
O_B0 = 512
O_B1 = 768
O_WPT = 1024
O_WLB = 1151
O_BNA = 1278
O_BNB = 1279
O_ALP = 1280
O_SEL = 1281
CBLK_W = 1537


def host_consts(conv_w, bn_gamma, bn_beta, bn_mean, bn_var, eca_w):
    w = np.asarray(eca_w, np.float64).reshape(5)
    conv_w = np.asarray(conv_w, np.float64)          # [127, 129]
    id128 = np.eye(HALF)
    tri = np.triu(np.ones((HALF, HALF)), 1)          # tri[k, j] = 1 iff k < j
    ones = np.ones((HALF, HALF))
    iota = np.tile(np.arange(HALF, dtype=np.float64), (HALF, 1))
    Bm = np.zeros((2, HALF, C))
    for h in range(2):
        for k in range(HALF):
            c = h * HALF + k
            for t in range(5):
                cp = c - t + 2
                if 0 <= cp < C:
                    Bm[h, k, cp] = w[t]
    wposT = conv_w[:, :HALF].T                        # [128, 127]
    wlastb = np.tile(conv_w[:, HALF][None, :], (HALF, 1))
    a = np.asarray(bn_gamma, np.float64) / np.sqrt(
        np.asarray(bn_var, np.float64) + BN_EPS)
    bnA = np.zeros((HALF, 1)); bnA[0, 0] = 1.0; bnA[1:, 0] = a
    bnB = np.zeros((HALF, 1))
    bnB[1:, 0] = (np.asarray(bn_beta, np.float64)
                  - np.asarray(bn_mean, np.float64) * a)
    alpha = np.full((HALF, 1), 0.1); alpha[0, 0] = 1.0
    sel = np.zeros((HALF, 2 * HALF))
    sel[0, 0:HALF] = 1.0
    sel[1, HALF:2 * HALF] = 1.0
    cblk = np.concatenate(
        [id128, tri, ones, iota, Bm[0], Bm[1], wposT, wlastb, bnA, bnB, alpha,
         sel], axis=1).astype(np.float32)
    assert cblk.shape == (HALF, CBLK_W)
    import ml_dtypes
    cbf = np.concatenate([id128, wposT], axis=1).astype(ml_dtypes.bfloat16)
    return {"cblk": cblk, "cbf": cbf}


def build_nc(reps=1, npix=NPIX, nsamp=SPC, mh0_engine="vector"):
    nc = bass.Bass("TRN2", target_bir_lowering=False, debug=False)

    x0 = nc.dram_tensor("x0", [nsamp, C, npix], F32, kind="ExternalInput").ap()
    x1 = nc.dram_tensor("x1", [nsamp, C, npix], F32, kind="ExternalInput").ap()
    cbd = nc.dram_tensor("cblk", [HALF, CBLK_W], F32, kind="ExternalInput").ap()
    cbfd = nc.dram_tensor("cbf", [HALF, 255], BF16, kind="ExternalInput").ap()
    out = nc.dram_tensor("out", [nsamp, 2 * C, npix], F32,
                         kind="ExternalOutput").ap()

    AL = mybir.AluOpType
    from contextlib import ExitStack
    with tile.TileContext(nc) as tc, ExitStack() as st:
        consts = st.enter_context(tc.tile_pool(name="consts", bufs=1))
        xin = st.enter_context(tc.tile_pool(name="xin", bufs=1))
        lhp = st.enter_context(tc.tile_pool(name="lhp", bufs=1))
        misc = st.enter_context(tc.tile_pool(name="misc", bufs=1))
        epi = st.enter_context(tc.tile_pool(name="epi", bufs=2))
        obp = st.enter_context(tc.tile_pool(name="obp", bufs=2))
        prk = st.enter_context(tc.tile_pool(name="prk", bufs=3, space="PSUM"))
        pbig = st.enter_context(tc.tile_pool(name="pbig", bufs=2, space="PSUM"))

        cb = consts.tile([HALF, CBLK_W], F32)
        nc.sync.dma_start(out=cb, in_=cbd)
        cbf = consts.tile([HALF, 255], BF16)
        nc.sync.dma_start(out=cbf, in_=cbfd)
        c_id16 = cbf[:, 0:128]
        c_wposT16 = cbf[:, 128:255]
        c_id = cb[:, O_ID:O_ID + 128]
        c_id2 = cb[0:2, O_ID:O_ID + 2]
        c_tri = cb[:, O_TRI:O_TRI + 128]
        c_ones = cb[:, O_ONES:O_ONES + 128]
        c_iota = cb[:, O_IOTA:O_IOTA + 128]
        c_B = [cb[:, O_B0:O_B0 + C], cb[:, O_B1:O_B1 + C]]
        c_wposT = cb[:, O_WPT:O_WPT + 127]
        c_wlastb = cb[:, O_WLB:O_WLB + 127]
        c_bnA = cb[:, O_BNA:O_BNA + 1]
        c_bnB = cb[:, O_BNB:O_BNB + 1]
        c_alpha = cb[:, O_ALP:O_ALP + 1]

        for rep in range(reps):
            # ---- loads (f32 bounce) + fused bf16 cast + channel sums ----
            X = [[None, None] for _ in range(nsamp)]
            SM = misc.tile([HALF, 4], F32, tag="SM")
            for s in range(nsamp):
                for h in range(2):
                    t = xin.tile([HALF, npix], F32, tag="xt", bufs=2)
                    nc.sync.dma_start(out=t,
                                      in_=x1[s, h * HALF:(h + 1) * HALF, :])
                    xb = xin.tile([HALF, npix], BF16, tag=f"xb_{s}_{h}")
                    nc.vector.tensor_scalar(
                        out=xb, in0=t, scalar1=1.0, scalar2=None,
                        op0=AL.mult, op1=AL.add,
                        accum_out=SM[:, h * 2 + s:h * 2 + s + 1])
                    X[s][h] = xb

            # ---- scores y (GAP+ECA fused): Y2 [2, 256] = sum_h SM_h^T @ B_h ----
            Y2 = prk.tile([2, C], F32, tag="mp")
            nc.tensor.matmul(Y2, SM[:, 0:2], c_B[0], start=True, stop=False)
            nc.tensor.matmul(Y2, SM[:, 2:4], c_B[1], start=False, stop=True)
            y_sb = misc.tile([2, C], F32, tag="ysb")
            nc.vector.tensor_copy(out=y_sb, in_=Y2)

            # ---- yT [128, 4]: y with channel-as-partition ----
            pyT = prk.tile([HALF, 4], F32, tag="mp")
            for h in range(2):
                nc.tensor.matmul(pyT[:, h * 2:h * 2 + 2],
                                 y_sb[:, h * HALF:(h + 1) * HALF], c_id2,
                                 start=True, stop=True)
            ycT = misc.tile([HALF, 4], F32, tag="ycT")
            nc.vector.tensor_copy(out=ycT, in_=pyT)

            # ---- broadcast y along partitions: pbY [128, 512] ----
            pbY = prk.tile([HALF, 2 * C], F32, tag="mp")
            for s in range(nsamp):
                nc.tensor.matmul(pbY[:, s * C:(s + 1) * C],
                                 cb[0:2, O_SEL + s * HALF:O_SEL + (s + 1) * HALF],
                                 y_sb, start=True, stop=True)

            # ---- ranks RD[:, h*2+s] = #{c' : y[c'] > y[c]} ----
            RD = misc.tile([HALF, 4], F32, tag="RD")
            for s in range(nsamp):
                for h in range(2):
                    junk = misc.tile([HALF, C], F32, tag="junk", bufs=2)
                    nc.vector.tensor_scalar(
                        out=junk, in0=pbY[:, s * C:(s + 1) * C],
                        scalar1=ycT[:, h * 2 + s:h * 2 + s + 1], scalar2=None,
                        op0=AL.is_gt, op1=AL.add,
                        accum_out=RD[:, h * 2 + s:h * 2 + s + 1])

            # ---- masks ----
            M = misc.tile([HALF, 4], F32, tag="M")
            ND = misc.tile([HALF, 4], F32, tag="ND")
            nc.vector.tensor_scalar(out=M, in0=RD, scalar1=float(HALF),
                                    scalar2=None, op0=AL.is_lt)
            nc.vector.tensor_scalar(out=ND, in0=RD, scalar1=float(HALF),
                                    scalar2=1.0 / HALF, op0=AL.is_ge,
                                    op1=AL.mult)

            # ---- positions P; RP = 32768*ND + P ----
            P = prk.tile([HALF, 4], F32, tag="mp")
            nc.tensor.matmul(P[:, 0:2], c_tri, M[:, 0:2], start=True, stop=True)
            nc.tensor.matmul(P[:, 2:4], c_tri, M[:, 2:4], start=True, stop=False)
            nc.tensor.matmul(P[:, 2:4], c_ones, M[:, 0:2], start=False,
                             stop=True)
            RP = misc.tile([HALF, 4], F32, tag="RP")
            nc.vector.scalar_tensor_tensor(out=RP, in0=ND, scalar=32768.0,
                                           in1=P, op0=AL.mult, op1=AL.add)

            # ---- G columns + negdiv column of LHS ----
            LHS = [[None, None] for _ in range(nsamp)]
            for s in range(nsamp):
                for h in range(2):
                    lh = lhp.tile([HALF, C], BF16, tag=f"lh_{s}_{h}")
                    LHS[s][h] = lh
                    nc.vector.tensor_scalar(
                        out=lh[:, 0:HALF], in0=c_iota,
                        scalar1=RP[:, h * 2 + s:h * 2 + s + 1], scalar2=None,
                        op0=AL.is_equal)
                    nc.vector.tensor_copy(out=lh[:, HALF:HALF + 1],
                                          in_=ND[:, h * 2 + s:h * 2 + s + 1])

            # ---- W columns: transpose G, multiply by W_pos^T, assemble ----
            sh_pairs = [(s, h) for s in range(nsamp) for h in range(2)]
            pgm = prk.tile([HALF, 4 * HALF], F32, tag="mp")
            for i, (s, h) in enumerate(sh_pairs):
                nc.tensor.matmul(pgm[:, i * HALF:(i + 1) * HALF],
                                 LHS[s][h][:, 0:HALF], c_id16,
                                 start=True, stop=True)
            gm_all = misc.tile([HALF, 4 * HALF], BF16, tag="gm")
            nc.vector.tensor_copy(out=gm_all, in_=pgm)
            pwg = prk.tile([HALF, 4 * HALF], F32, tag="mp")
            for i, (s, h) in enumerate(sh_pairs):
                nc.tensor.matmul(pwg[:, i * HALF:i * HALF + 127],
                                 gm_all[:, i * HALF:(i + 1) * HALF], c_wposT16,
                                 start=True, stop=True)
            for i, (s, h) in enumerate(sh_pairs):
                nc.vector.scalar_tensor_tensor(
                    out=LHS[s][h][:, HALF + 1:C], in0=c_wlastb,
                    scalar=ND[:, h * 2 + s:h * 2 + s + 1],
                    in1=pwg[:, i * HALF:i * HALF + 127],
                    op0=AL.mult, op1=AL.add)

            # ---- big matmuls + epilogue + stores ----
            grps = []
            g0 = 0
            while g0 < npix:
                grps.append((g0, min(GRP, npix - g0)))
                g0 += GRP
            for s in range(nsamp):
                for mh in range(2):
                    ob = obp.tile([HALF, npix], F32, tag="ob")
                    for (g0, gsz) in grps:
                        ps = pbig.tile([HALF, GRP], F32, tag="pb")
                        n0 = 0
                        while n0 < gsz:
                            nsz = min(NT, gsz - n0)
                            for h in range(2):
                                nc.tensor.matmul(
                                    ps[:, n0:n0 + nsz],
                                    LHS[s][h][:, mh * HALF:(mh + 1) * HALF],
                                    X[s][h][:, g0 + n0:g0 + n0 + nsz],
                                    start=(h == 0), stop=(h == 1))
                            n0 += nsz
                        if mh == 0:
                            getattr(nc, mh0_engine).tensor_add(
                                out=ob[:, g0:g0 + gsz], in0=ps[:, :gsz],
                                in1=X[s][0][:, g0:g0 + gsz])
                        else:
                            q = epi.tile([HALF, GRP], F32, tag="q")
                            nc.scalar.activation(
                                out=q[:, :gsz], in_=ps[:, :gsz],
                                func=mybir.ActivationFunctionType.Identity,
                                bias=c_bnB, scale=c_bnA)
                            m = epi.tile([HALF, GRP], F32, tag="m")
                            nc.vector.scalar_tensor_tensor(
                                out=m[:, :gsz], in0=q[:, :gsz], scalar=c_alpha,
                                in1=q[:, :gsz], op0=AL.mult, op1=AL.max)
                            nc.vector.tensor_add(out=ob[:, g0:g0 + gsz],
                                                 in0=m[:, :gsz],
                                                 in1=X[s][1][:, g0:g0 + gsz])
                    nc.sync.dma_start(
                        out=out[s, C + mh * HALF:C + (mh + 1) * HALF, :],
                        in_=ob)
                # x0 passthrough DRAM->DRAM, after this sample's stores
                nc.sync.dma_start(out=out[s, 0:C, :], in_=x0[s, :, :])
    return nc


def _split_multiwait_drains(nc):
    """This container's walrus rejects >1 sync-wait on one instruction -
    split Tile's kernel-tail multi-wait Drains into single-wait chains."""
    for fn in nc.m.functions:
        for blk in fn.blocks:
            insts = list(blk.instructions)
            changed = False
            outl = []
            for inst in insts:
                si = getattr(inst, "sync_info", None)
                waits = list(si.on_wait) if (si and si.on_wait) else []
                if len(waits) > 1:
                    for j, w in enumerate(waits[:-1]):
                        nd = mybir.InstEventSemaphore(
                            name=f"{inst.name}-sw{j}", ins=[], outs=[])
                        nd.engine = inst.engine
                        nd.sync_info = mybir.SyncInfo(on_wait=[w], on_update=[])
                        outl.append(nd)
                    si.on_wait = [waits[-1]]
                    changed = True
                outl.append(inst)
            if changed:
                blk.instructions = outl
    return nc


def kernel(x0, x1, eca_w, conv_w, bn_gamma, bn_beta, bn_mean, bn_var):
    x0 = np.asarray(x0, np.float32).reshape(B, C, NPIX)
    x1 = np.asarray(x1, np.float32).reshape(B, C, NPIX)
    cst = host_consts(conv_w, bn_gamma, bn_beta, bn_mean, bn_var, eca_w)
    nc = _split_multiwait_drains(build_nc())
    in_maps = []
    for c in range(NCORES):
        m = dict(cst)
        m["x0"] = np.ascontiguousarray(x0[c * SPC:(c + 1) * SPC])
        m["x1"] = np.ascontiguousarray(x1[c * SPC:(c + 1) * SPC])
        in_maps.append(m)
    res = run_bass_kernel_spmd(nc, in_maps, list(range(NCORES)), trace=False)
    out = np.concatenate([res.results[c]["out"] for c in range(NCORES)], axis=0)
    return out.reshape(B, 2 * C, H, W)
